# revision 1
# baseline (speedup 1.0000x reference)
"""Trainium2 Bass kernel for nn_BaseAttention (B=2,S=2048,D=1024,H=16,K=64).

Sharding: 8 cores = (batch b in {0,1}) x (query-block qb in {0..3}, 512 rows).
Each core computes K/V projections for the FULL sequence of its batch
(redundant across the 4 cores sharing a batch -- avoids any collective),
attention + output projection + residual + LayerNorm for its 512 query rows.
Host concatenates the 8 [512, 1024] output blocks.

Layouts (per core):
  xT  [D, S]   : x[b] transposed (host-side) -> contraction dim d on partitions
  QT/KT [hk,s] : produced transposed, feeds scoresT = K^T(64) x Q^T directly
  scoresT [s,q]: softmax denominator obtained via an all-ones 65th column
                 appended to V (ctx matmul also emits sum_s exp = denom row)
  ctxT [hk, q] : feeds output projection with wo [hk, d] in natural layout
All big matmuls run in float32r (fp22 truncated fp32, full PE rate at N>=256).
"""

import sys
import numpy as np

B, S, D, H, KD = 2, 2048, 1024, 16, 64
SB = S // 4
HK = H * KD
P = 128

if "/opt/trn_rl_repo" not in sys.path:
    sys.path.insert(0, "/opt/trn_rl_repo")

_cache = {}


def _build(D_, S_, SB_, H_):
    import concourse.bass as bass
    import concourse.mybir as mybir
    from concourse.tile import TileContext

    dt = mybir.dt
    f32, f32r = dt.float32, dt.float32r
    AF = mybir.ActivationFunctionType
    OP = mybir.AluOpType
    AX = mybir.AxisListType.X

    HK_ = H_ * KD
    DC = D_ // P              # d-chunks of 128
    NSC = S_ // P             # s-chunks of 128
    S5 = min(512, S_)
    NS5 = S_ // S5            # s-chunks of 512 (moving dim)
    QN = SB_                  # scores moving width (<=512)
    NQC = SB_ // P
    D5 = min(512, D_)
    ND5 = D_ // D5
    NG = H_ // 4 if H_ >= 8 else 2    # head groups (SBUF residency)
    GH = H_ // NG
    GW = GH * KD              # group width in hk (<=512)
    NKT = GW // P             # KT tiles per group
    HC = HK_ // P
    assert QN <= 512 and GW <= 512

    nc = bass.Bass()
    xT = nc.declare_dram_parameter("xT", [D_, S_], f32r, isOutput=False)
    xqT = nc.declare_dram_parameter("xqT", [D_, SB_], f32r, isOutput=False)
    xq = nc.declare_dram_parameter("xq", [SB_, D_], f32, isOutput=False)
    wq = nc.declare_dram_parameter("wq", [D_, HK_], f32r, isOutput=False)
    wk = nc.declare_dram_parameter("wk", [D_, HK_], f32r, isOutput=False)
    wv = nc.declare_dram_parameter("wv", [D_, HK_], f32r, isOutput=False)
    wo = nc.declare_dram_parameter("wo", [HK_, D_], f32r, isOutput=False)
    bqp = nc.declare_dram_parameter("bqT", [P, HC], f32, isOutput=False)
    bkp = nc.declare_dram_parameter("bkT", [P, HC], f32, isOutput=False)
    bvp = nc.declare_dram_parameter("bv_row", [1, HK_], f32r, isOutput=False)
    bop = nc.declare_dram_parameter("bo_row", [1, D_], f32, isOutput=False)
    gmp = nc.declare_dram_parameter("gamma_row", [1, D_], f32, isOutput=False)
    btp = nc.declare_dram_parameter("beta_row", [1, D_], f32, isOutput=False)
    out = nc.declare_dram_parameter("out", [SB_, D_], f32, isOutput=True)

    with TileContext(nc) as tc:
        with tc.tile_pool(name="const", bufs=1) as cpool, \
             tc.tile_pool(name="ctxn", bufs=H_) as cnp:

            ones = cpool.tile([128, P], f32, tag="ones")
            nc.vector.memset(ones[:], 1.0)
            ones_r32 = cpool.tile([1, P], f32, tag="ones_r")
            nc.vector.memset(ones_r32[:], 1.0)
            ones_r = ones_r32[:].bitcast(f32r)
            eps_t = cpool.tile([P, 1], f32, tag="eps")
            nc.vector.memset(eps_t[:], 1e-6)
            # DMA order below is load-bearing: HW DMA queues are assigned
            # round-robin in scheduled order, and blocks of 8 tile-loads are
            # kept queue-aligned so each matmul's inputs share one queue
            # (walrus allows only one sync-wait on f32r matmuls).
            bq_sb = cpool.tile([P, HC], f32, tag="bq")
            nc.sync.dma_start(out=bq_sb[:], in_=bqp[:])
            bk_sb = cpool.tile([P, HC], f32, tag="bk")
            nc.sync.dma_start(out=bk_sb[:], in_=bkp[:])
            bv_sb = cpool.tile([1, HK_], f32r, tag="bv")
            nc.sync.dma_start(out=bv_sb[:], in_=bvp[:])

            # x^T resident for all projections (released before out-proj)
            xtq_scope = tc.tile_pool(name="xT", bufs=DC)
            xtp = xtq_scope.__enter__()
            qt_scope = tc.tile_pool(name="QT", bufs=HC)
            qtp = qt_scope.__enter__()
            xt_sb = []
            for dc in range(DC):
                t = xtp.tile([P, S_], f32r, tag="xt")
                nc.sync.dma_start(out=t[:], in_=xT[dc * P:(dc + 1) * P, :])
                xt_sb.append(t)

            # ---- Q^T [hk, q] = wq^T x xqT, + bias, x 1/sqrt(K) ----
            qt_sb = []
            with tc.tile_pool(name="wq", bufs=DC) as wqp, \
                 tc.tile_pool(name="xqT", bufs=DC) as xqp, \
                 tc.tile_pool(name="qps", bufs=2, space="PSUM") as qps:
                wq_sb, xq_sb = [], []
                for dc in range(DC):
                    t = wqp.tile([P, HK_], f32r, tag="wq")
                    nc.sync.dma_start(out=t[:], in_=wq[dc * P:(dc + 1) * P, :])
                    wq_sb.append(t)
                for dc in range(DC):
                    t = xqp.tile([P, SB_], f32r, tag="xq")
                    nc.sync.dma_start(out=t[:], in_=xqT[dc * P:(dc + 1) * P, :])
                    xq_sb.append(t)
                for t in range(HC):
                    pt = qps.tile([P, QN], f32, tag="qps")
                    for dc in range(DC):
                        nc.tensor.matmul(pt[:], wq_sb[dc][:, t * P:(t + 1) * P],
                                         xq_sb[dc][:],
                                         start=(dc == 0), stop=(dc == DC - 1))
                    q_t = qtp.tile([P, QN], f32r, tag="qt")
                    nc.vector.tensor_scalar(q_t[:], pt[:], bq_sb[:, t:t + 1],
                                            1.0 / np.sqrt(KD), OP.add, OP.mult)
                    qt_sb.append(q_t)

            ctxn = []
            for g in range(NG):
                with tc.tile_pool(name="ktg", bufs=NKT) as ktpool:
                    # ---- K^T group [GW, S] ----
                    kt_sb = []
                    with tc.tile_pool(name="wk", bufs=DC) as wkp, \
                         tc.tile_pool(name="kps", bufs=1, space="PSUM") as kps:
                        wk_sb = []
                        for dc in range(DC):
                            t = wkp.tile([P, GW], f32r, tag="wk")
                            nc.sync.dma_start(
                                out=t[:],
                                in_=wk[dc * P:(dc + 1) * P, g * GW:(g + 1) * GW])
                            wk_sb.append(t)
                        for t in range(NKT):
                            pt = kps.tile([P, S_], f32, tag="kps")
                            for dc in range(DC):
                                for s5 in range(NS5):
                                    nc.tensor.matmul(
                                        pt[:, s5 * S5:(s5 + 1) * S5],
                                        wk_sb[dc][:, t * P:(t + 1) * P],
                                        xt_sb[dc][:, s5 * S5:(s5 + 1) * S5],
                                        start=(dc == 0), stop=(dc == DC - 1))
                            kt_t = ktpool.tile([P, S_], f32r, tag="kt")
                            nc.vector.tensor_scalar(
                                kt_t[:], pt[:],
                                bk_sb[:, (g * NKT + t):(g * NKT + t) + 1],
                                None, OP.add)
                            kt_sb.append(kt_t)

                    # ---- attention, V produced just-in-time per s-chunk ----
                    with tc.tile_pool(name="wv", bufs=DC) as wvp, \
                         tc.tile_pool(name="vaug", bufs=3) as vaugp, \
                         tc.tile_pool(name="exp", bufs=3) as epool, \
                         tc.tile_pool(name="rdp", bufs=2) as rdpool, \
                         tc.tile_pool(name="rbp", bufs=2) as rbpool, \
                         tc.tile_pool(name="vps", bufs=2, space="PSUM") as vps, \
                         tc.tile_pool(name="sps", bufs=2, space="PSUM") as sps, \
                         tc.tile_pool(name="cps", bufs=GH, space="PSUM") as cps:
                        wv_sb = []
                        for dc in range(DC):
                            t = wvp.tile([P, GW], f32r, tag="wv")
                            nc.sync.dma_start(
                                out=t[:],
                                in_=wv[dc * P:(dc + 1) * P, g * GW:(g + 1) * GW])
                            wv_sb.append(t)
                        pc = [cps.tile([P, QN], f32, tag="cps",
                                       name=f"pc{g}_{i}")
                              for i in range(GH)]
                        for i in range(GH):
                            # ACT write absorbs the slot-release wait so the
                            # first ctx matmul only waits on ACT
                            nc.scalar.activation(pc[i][0:1, 0:2],
                                                 bq_sb[0:1, 0:2], AF.Copy,
                                                 scale=0.0)
                        for sc in range(NSC):
                            pv = vps.tile([P, GW], f32, tag="vps")
                            for dc in range(DC):
                                nc.tensor.matmul(
                                    pv[:], xt_sb[dc][:, sc * P:(sc + 1) * P],
                                    wv_sb[dc][:],
                                    start=(dc == 0), stop=False)
                            # + bv broadcast via K=1 matmul (keeps va ACT-only)
                            nc.tensor.matmul(
                                pv[:], ones_r[0:1, 0:P],
                                bv_sb[0:1, g * GW:(g + 1) * GW],
                                start=False, stop=True)
                            va = vaugp.tile([P, GH * 65], f32r, tag="va")
                            vav = va[:].rearrange("p (h k) -> p h k", k=65)
                            nc.scalar.copy(
                                vav[:, :, 0:64],
                                pv[:].rearrange("p (h k) -> p h k", k=64))
                            nc.scalar.activation(
                                vav[:, :, 64:65],
                                pv[:, 0:GH].rearrange("p (h o) -> p h o", o=1),
                                AF.Copy, bias=1.0, scale=0.0)
                            for hl in range(GH):
                                h = g * GH + hl
                                po = (hl * KD) % P
                                qtile = qt_sb[(h * KD) // P]
                                qpo = (h * KD) % P
                                ps = sps.tile([P, QN], f32, tag="sps")
                                nc.tensor.matmul(
                                    ps[:],
                                    kt_sb[(hl * KD) // P][po:po + KD,
                                                          sc * P:(sc + 1) * P],
                                    qtile[qpo:qpo + KD, :],
                                    start=True, stop=True)
                                et = epool.tile([P, QN], f32r, tag="exp")
                                nc.scalar.activation(et[:], ps[:], AF.Exp)
                                nc.tensor.matmul(
                                    pc[hl][0:65, :],
                                    va[:, hl * 65:(hl + 1) * 65],
                                    et[:],
                                    start=(sc == 0), stop=(sc == NSC - 1))
                        # normalize: ctxT[0:64] * (1/denom row 64) bcast
                        for hl in range(GH):
                            rd = rdpool.tile([65, QN], f32, tag="rd")
                            nc.vector.reciprocal(rd[64:65, :], pc[hl][64:65, :])
                            prb = sps.tile([64, QN], f32, tag="sps")
                            nc.tensor.matmul(prb[:], ones[64:65, 0:64],
                                             rd[64:65, :], start=True,
                                             stop=True)
                            rb = rbpool.tile([64, QN], f32r, tag="rb")
                            nc.vector.tensor_copy(rb[:], prb[:])
                            cn = cnp.tile([64, QN], f32r, tag="cn")
                            nc.vector.tensor_tensor(cn[:], pc[hl][0:64, :],
                                                    rb[:], OP.mult)
                            ctxn.append(cn)

            # release x^T and Q^T before the out-projection phase
            qt_scope.__exit__(None, None, None)
            xtq_scope.__exit__(None, None, None)

            # ---- output projection + residual + LayerNorm ----
            with tc.tile_pool(name="wo", bufs=H_) as wop, \
                 tc.tile_pool(name="lnB", bufs=1) as lbp, \
                 tc.tile_pool(name="xq2", bufs=2) as xqp2, \
                 tc.tile_pool(name="ln", bufs=2) as lnp, \
                 tc.tile_pool(name="st", bufs=8) as stp, \
                 tc.tile_pool(name="ops", bufs=2, space="PSUM") as ops:
                wo_sb = []
                for h in range(H_):
                    t = wop.tile([KD, D_], f32r, tag="wo")
                    nc.sync.dma_start(out=t[:], in_=wo[h * KD:(h + 1) * KD, :])
                    wo_sb.append(t)
                boB = lbp.tile([P, D_], f32, tag="boB")
                gmB = lbp.tile([P, D_], f32, tag="gmB")
                btB = lbp.tile([P, D_], f32, tag="btB")
                nc.sync.dma_start(out=boB[:], in_=bop[:].to_broadcast((P, D_)))
                nc.sync.dma_start(out=gmB[:], in_=gmp[:].to_broadcast((P, D_)))
                nc.sync.dma_start(out=btB[:], in_=btp[:].to_broadcast((P, D_)))

                # PE observes the last attn DVE tick once, so the first
                # real out-proj matmul only waits on its wo DMA queue
                obs = ops.tile([1, 2], f32, tag="ops", name="obs")
                nc.tensor.matmul(obs[:], ctxn[H_ - 1][:, 0:1],
                                 ctxn[H_ - 1][:, 0:2], start=True, stop=True)
                for qc in range(NQC):
                    po_ = ops.tile([P, D_], f32, tag="ops")
                    for h in range(H_):
                        for d5 in range(ND5):
                            nc.tensor.matmul(
                                po_[:, d5 * D5:(d5 + 1) * D5],
                                ctxn[h][:, qc * P:(qc + 1) * P],
                                wo_sb[h][:, d5 * D5:(d5 + 1) * D5],
                                start=(h == 0), stop=(h == H_ - 1))
                    xq_t = xqp2.tile([P, D_], f32, tag="xq2")
                    nc.sync.dma_start(out=xq_t[:], in_=xq[qc * P:(qc + 1) * P, :])
                    y = lnp.tile([P, D_], f32, tag="y")
                    nc.vector.tensor_tensor(y[:], po_[:], xq_t[:], OP.add)
                    nc.vector.tensor_tensor(y[:], y[:], boB[:], OP.add)
                    sum_t = stp.tile([P, 1], f32, tag="sum")
                    nc.vector.reduce_sum(out=sum_t[:], in_=y[:], axis=AX)
                    mean_t = stp.tile([P, 1], f32, tag="mean")
                    nc.vector.tensor_scalar_mul(mean_t[:], sum_t[:], 1.0 / D_)
                    cent = lnp.tile([P, D_], f32, tag="cent")
                    nc.vector.tensor_scalar(cent[:], y[:], mean_t[:], None,
                                            OP.subtract)
                    sq = lnp.tile([P, D_], f32, tag="sq")
                    vs = stp.tile([P, 1], f32, tag="vs")
                    nc.scalar.activation(sq[:], cent[:], AF.Square,
                                         accum_out=vs[:])
                    std = stp.tile([P, 1], f32, tag="std")
                    nc.scalar.activation(std[:], vs[:], AF.Sqrt,
                                         bias=eps_t[:], scale=1.0 / D_)
                    rstd = stp.tile([P, 1], f32, tag="rstd")
                    nc.vector.reciprocal(rstd[:], std[:])
                    nrm = lnp.tile([P, D_], f32, tag="nrm")
                    nc.vector.tensor_scalar_mul(nrm[:], cent[:], rstd[:])
                    ot = lnp.tile([P, D_], f32, tag="ot")
                    nc.vector.tensor_tensor(ot[:], nrm[:], gmB[:], OP.mult)
                    nc.vector.tensor_tensor(ot[:], ot[:], btB[:], OP.add)
                    nc.sync.dma_start(out=out[qc * P:(qc + 1) * P, :], in_=ot[:])

    # Post-pass: walrus's per-instruction ISA structs hold only ONE sync
    # wait for compute-engine instructions (S3_LW for matmul, S3D3_TS for
    # tensor_scalar, ...). Move excess waits onto standalone
    # EventSemaphore instructions placed just before on the same engine
    # stream (sequencer executes them in order; semantics unchanged).
    SPLIT = {"InstMatmult", "InstTensorScalarPtr", "InstTensorScalar",
             "InstTensorTensor", "InstReciprocal", "InstActivation",
             "InstTensorReduce", "InstTensorCopy", "InstMemSet",
             "InstCopy", "InstDMACopy", "InstDMATranspose", "InstDrain"}
    evt_n = 0
    for f in nc.m.functions:
        for bb in f.blocks:
            need = any(
                type(i).__name__ in SPLIT and i.sync_info is not None
                and len(i.sync_info.on_wait) > 1 for i in bb.instructions)
            if not need:
                continue
            newl = []
            for ins in bb.instructions:
                si = ins.sync_info
                if (type(ins).__name__ in SPLIT and si is not None
                        and len(si.on_wait) > 1):
                    extra = list(si.on_wait[:-1])
                    for j in range(0, len(extra), 2):  # evt-sem holds <=2
                        evt_n += 1
                        evt = mybir.InstEventSemaphore(name=f"mmwait_{evt_n}")
                        evt.engine = ins.engine
                        evt.sync_info = mybir.SyncInfo(
                            on_wait=extra[j:j + 2], on_update=[])
                        newl.append(evt)
                    ins.sync_info = mybir.SyncInfo(
                        on_wait=[si.on_wait[-1]],
                        on_update=list(si.on_update))
                newl.append(ins)
            bb.instructions = newl
    return nc


def get_nc(D_=D, S_=S, SB_=SB, H_=H):
    key = (D_, S_, SB_, H_)
    if key not in _cache:
        _cache[key] = _build(D_, S_, SB_, H_)
    return _cache[key]


def make_in_maps(inputs, D_=D, S_=S, SB_=SB, H_=H, n_cores=8):
    """Shard full inputs into per-core input maps (host-side layout prep)."""
    HK_ = H_ * KD
    HC = HK_ // P
    nb = inputs["x"].shape[0]
    nq = n_cores // nb
    f = np.float32
    wq_ = np.ascontiguousarray(inputs["wq"].reshape(D_, HK_), f)
    wk_ = np.ascontiguousarray(inputs["wk"].reshape(D_, HK_), f)
    wv_ = np.ascontiguousarray(inputs["wv"].reshape(D_, HK_), f)
    wo_ = np.ascontiguousarray(inputs["wo"].reshape(HK_, D_), f)
    bqT = np.ascontiguousarray(np.asarray(inputs["bq"], f).reshape(HC, P).T)
    bkT = np.ascontiguousarray(np.asarray(inputs["bk"], f).reshape(HC, P).T)
    bv_row = np.asarray(inputs["bv"], f).reshape(1, HK_)
    bo_row = np.asarray(inputs["bo"], f).reshape(1, D_)
    gm_row = np.asarray(inputs["gamma"], f).reshape(1, D_)
    bt_row = np.asarray(inputs["beta"], f).reshape(1, D_)
    maps = []
    for c in range(n_cores):
        b, qb = c // nq, c % nq
        xb = np.asarray(inputs["x"][b], f)
        xTb = np.ascontiguousarray(xb.T)
        maps.append(dict(
            xT=xTb,
            xqT=np.ascontiguousarray(xTb[:, qb * SB_:(qb + 1) * SB_]),
            xq=np.ascontiguousarray(xb[qb * SB_:(qb + 1) * SB_]),
            wq=wq_, wk=wk_, wv=wv_, wo=wo_,
            bqT=bqT, bkT=bkT, bv_row=bv_row, bo_row=bo_row,
            gamma_row=gm_row, beta_row=bt_row,
        ))
    return maps


def kernel(**inputs):
    from concourse.bass_utils import run_bass_kernel_spmd
    nc = get_nc()
    maps = make_in_maps(inputs)
    res = run_bass_kernel_spmd(nc, maps, list(range(8)))
    x = inputs["x"]
    outp = np.empty((B, S, D), np.float32)
    nq = 8 // B
    for c in range(8):
        b, qb = c // nq, c % nq
        outp[b, qb * SB:(qb + 1) * SB] = res.results[c]["out"]
    return outp



# revision 18
# speedup vs baseline: 2.7506x; 2.7506x over previous
"""Trainium2 Bass kernel for nn_BaseAttention (B=2,S=2048,D=1024,H=16,K=64).

Sharding v2: 8 cores = (batch b in {0,1}) x (head-group g in {0..3}, 4 heads).
Each core computes Q/K/V projections for its 4 heads over the FULL sequence,
attention for its 4 heads (all 2048 q rows), then an AllToAll within each
4-core batch group redistributes context so core (b,g) holds ALL 16 heads for
q-block g (512 rows). Output projection + residual + LayerNorm on that block.
No redundant compute; the only collective is a 1MB AllToAll of bf16 context.

Per-core engine plan:
  PE    : all matmuls in bf16 (1 cycle/row vs 1.5 for f32r).
          - scores:   per head-pair, row-tiled (heads at partitions 0-63 /
            64-127 -> tile_position rows 0/64) so two 64-contraction matmuls
            run concurrently in the 128x128 array.
          - context:  per head-pair, col-tiled (outputs at PSUM partitions
            0-63 / 64-127) -> concurrent.
          - denoms:   sum_s exp via ones-column stationary [128,1], col-tiled
            4 ways at PSUM partitions 0/32/64/96.
          - V natural layout obtained by PE-transposing V^T tiles.
  ACT   : exact exp (table) on a share of score tiles; LN square/sqrt.
  DVE   : Schraudolph bf16 exp (bitcast int16(x*184.665+B)) on the rest;
          PSUM->SBUF copies with bias; softmax normalize; LN elementwise.
  GPSIMD: triggers the AllToAll.
Scores are computed pre-scaled: wq is folded with (1/sqrt(64))*184.665 so the
DVE exp is a single tensor_scalar add, and ACT exp uses scale=1/184.665.
"""

import sys
import numpy as np

B, S, D, H, KD = 2, 2048, 1024, 16, 64
P = 128
GH = 4                 # heads per core
GHK = GH * KD          # 256
SB = S // 4            # 512 output rows per core
NQC = 4                # q chunks of 512
QW = 512
NSC = S // P           # 16 key chunks
DC = D // P            # 8 contraction chunks
NT = GHK // P          # 2 tiles (= head pairs) per core
HC = H * KD // P       # 8 hk tiles globally

EXPA = 184.6649652337873        # 2^7 / ln2
EXPB = 16250.65                 # Schraudolph bias (bf16), tuned numerically
ACT_FRAC_NUM, ACT_FRAC_DEN = 9, 16   # fraction of exp tiles on ACT engine

if "/opt/trn_rl_repo" not in sys.path:
    sys.path.insert(0, "/opt/trn_rl_repo")

_cache = {}


def _build():
    import concourse.bass as bass
    import concourse.mybir as mybir
    from concourse.tile import TileContext

    dt = mybir.dt
    f32, bf16, i16 = dt.float32, dt.bfloat16, dt.int16
    AF = mybir.ActivationFunctionType
    OP = mybir.AluOpType
    AX = mybir.AxisListType.X

    nc = bass.Bass()
    xT = nc.declare_dram_parameter("xT", [D, S], bf16, isOutput=False)
    wq = nc.declare_dram_parameter("wq", [D, GHK], bf16, isOutput=False)
    wk = nc.declare_dram_parameter("wk", [D, GHK], bf16, isOutput=False)
    wv = nc.declare_dram_parameter("wv", [D, GHK], bf16, isOutput=False)
    wo = nc.declare_dram_parameter("wo", [H * KD, D], bf16, isOutput=False)
    bqp = nc.declare_dram_parameter("bqT", [P, NT], f32, isOutput=False)
    bkp = nc.declare_dram_parameter("bkT", [P, NT], f32, isOutput=False)
    bvp = nc.declare_dram_parameter("bvT", [P, NT], f32, isOutput=False)
    xqb = nc.declare_dram_parameter("xqb", [SB, D], f32, isOutput=False)
    gmp = nc.declare_dram_parameter("gamma_row", [1, D], bf16, isOutput=False)
    btp = nc.declare_dram_parameter("beta_row", [1, D], f32, isOutput=False)
    idp = nc.declare_dram_parameter("ident", [P, P], bf16, isOutput=False)
    slp = nc.declare_dram_parameter("selm", [P, 2 * P], bf16, isOutput=False)
    out = nc.declare_dram_parameter("out", [SB, D], f32, isOutput=True)

    with TileContext(nc) as tc:
        with tc.tile_pool(name="const", bufs=1) as cpool, \
             tc.tile_pool(name="qk", bufs=NT) as qkp, \
             tc.tile_pool(name="va", bufs=NSC) as vap, \
             tc.tile_pool(name="wo", bufs=HC) as wop, \
             tc.tile_pool(name="cn", bufs=2 * NQC) as cnp, \
             tc.tile_pool(name="dram", bufs=2, space="DRAM") as drp:

            ones_bf = cpool.tile([P, 1], bf16, tag="ones")
            nc.vector.memset(ones_bf[:], 1.0)
            eps_t = cpool.tile([P, 1], f32, tag="eps")
            nc.vector.memset(eps_t[:], 1e-6)
            ident = cpool.tile([P, P], bf16, tag="ident")
            nc.sync.dma_start(out=ident[:], in_=idp[:])
            selm = cpool.tile([P, 2 * P], bf16, tag="selm")
            nc.sync.dma_start(out=selm[:], in_=slp[:])
            bq_sb = cpool.tile([P, NT], f32, tag="bq")
            nc.sync.dma_start(out=bq_sb[:], in_=bqp[:])
            bk_sb = cpool.tile([P, NT], f32, tag="bk")
            nc.sync.dma_start(out=bk_sb[:], in_=bkp[:])
            bv_sb = cpool.tile([P, NT], f32, tag="bv")
            nc.sync.dma_start(out=bv_sb[:], in_=bvp[:])
            gmB = cpool.tile([P, D], bf16, tag="gmB")
            nc.sync.dma_start(out=gmB[:], in_=gmp[:].to_broadcast((P, D)))
            btB = cpool.tile([P, D], f32, tag="btB")
            nc.sync.dma_start(out=btB[:], in_=btp[:].to_broadcast((P, D)))

            # AllToAll over all 8 cores: block j (rows 256j..256j+255) goes to
            # core j and holds this core's 4 heads (2 pair-tiles) for q rows
            # [256j, 256j+256) of this core's batch. Core j outputs those q
            # rows for BOTH batches, so every block is meaningful and all
            # offsets are batch-independent (pure SPMD).
            a2a_in = drp.tile([8 * 2 * P, 2 * P], bf16, tag="a2a_in")
            a2a_out = drp.tile([8 * 2 * P, 2 * P], bf16, tag="a2a_out")

            # ---- projections: Q^T, K^T, V^T [GHK, S], all bf16 ----
            qt_sb, kt_sb, vt_sb, va_sb = [], [], [], []
            vt_scope = tc.tile_pool(name="vt", bufs=NT)
            vtp = vt_scope.__enter__()
            with tc.tile_pool(name="xT", bufs=DC) as xtp, \
                 tc.tile_pool(name="w3", bufs=DC) as w3p, \
                 tc.tile_pool(name="pp", bufs=2, space="PSUM") as pp:
                xt_sb, wq_sb, wk_sb, wv_sb = [], [], [], []
                for dc in range(DC):
                    t = w3p.tile([P, GHK], bf16, tag="wq")
                    nc.sync.dma_start(out=t[:], in_=wq[dc * P:(dc + 1) * P, :])
                    wq_sb.append(t)
                for dc in range(DC):
                    t = xtp.tile([P, S], bf16, tag="xt")
                    nc.sync.dma_start(out=t[:], in_=xT[dc * P:(dc + 1) * P, :])
                    xt_sb.append(t)
                for dc in range(DC):
                    t = w3p.tile([P, GHK], bf16, tag="wk")
                    nc.sync.dma_start(out=t[:], in_=wk[dc * P:(dc + 1) * P, :])
                    wk_sb.append(t)
                for dc in range(DC):
                    t = w3p.tile([P, GHK], bf16, tag="wv")
                    nc.sync.dma_start(out=t[:], in_=wv[dc * P:(dc + 1) * P, :])
                    wv_sb.append(t)

                def project(w_sb, b_sb, out_list, pool, tag):
                    for t in range(NT):
                        pq = pp.tile([P, S], f32, tag="pp")
                        for dc in range(DC):
                            for m in range(S // QW):
                                nc.tensor.matmul(
                                    pq[:, m * QW:(m + 1) * QW],
                                    w_sb[dc][:, t * P:(t + 1) * P],
                                    xt_sb[dc][:, m * QW:(m + 1) * QW],
                                    start=(dc == 0), stop=(dc == DC - 1))
                        ot = pool.tile([P, S], bf16, tag=tag)
                        nc.vector.tensor_scalar(ot[:], pq[:], b_sb[:, t:t + 1],
                                                None, OP.add)
                        out_list.append(ot)

                project(wq_sb, bq_sb, qt_sb, qkp, "qt")
                project(wk_sb, bk_sb, kt_sb, qkp, "kt")
                project(wv_sb, bv_sb, vt_sb, vtp, "vt")
            # ---- V to natural layout [s, hk] via PE transpose ----
            with tc.tile_pool(name="tp", bufs=4, space="PSUM") as tpp:
                for sc in range(NSC):
                    pvt = tpp.tile([P, GHK], bf16, tag="tp")
                    with nc.allow_low_precision(
                            reason="pure transpose, no accumulation"):
                        for t in range(NT):
                            nc.tensor.transpose(
                                pvt[:, t * P:(t + 1) * P],
                                vt_sb[t][:, sc * P:(sc + 1) * P],
                                ident[:])
                    va = vap.tile([P, GHK], bf16, tag="va")
                    nc.vector.tensor_copy(va[:], pvt[:])
                    va_sb.append(va)
            vt_scope.__exit__(None, None, None)

            # wo / xq loads (needed late; queue after projection loads)
            wo_sb = []
            for hc in range(HC):
                t = wop.tile([P, D], bf16, tag="wo")
                nc.sync.dma_start(out=t[:], in_=wo[hc * P:(hc + 1) * P, :])
                wo_sb.append(t)
            xq_sb = []
            with tc.tile_pool(name="xq", bufs=4) as xqp:
                for qs in range(SB // P):
                    t = xqp.tile([P, D], f32, tag="xq")
                    nc.sync.dma_start(out=t[:], in_=xqb[qs * P:(qs + 1) * P, :])
                    xq_sb.append(t)

                # ---- attention ----
                exp_idx = 0
                with tc.tile_pool(name="ps", bufs=2, space="PSUM") as psp, \
                     tc.tile_pool(name="pc", bufs=3, space="PSUM") as pcp, \
                     tc.tile_pool(name="dn", bufs=1, space="PSUM") as dnp, \
                     tc.tile_pool(name="et", bufs=6) as etp, \
                     tc.tile_pool(name="dsb", bufs=2) as dsbp, \
                     tc.tile_pool(name="rb", bufs=2) as rbp:
                    for qc in range(NQC):
                        pc = [pcp.tile([P, QW], f32, tag="pc",
                                       name=f"pc{qc}_{i}") for i in range(NT)]
                        pdn = dnp.tile([P, QW], f32, tag="dn",
                                       name=f"pdn{qc}")
                        nc.vector.memset(pdn[:], 0.0)
                        ets = [None] * NSC  # per sc: [et_pair0, et_pair1] APs

                        def emit_scores(sc):
                            nonlocal exp_idx
                            pair_ets = []
                            for pr in range(NT):
                                ps = psp.tile([P, 2 * QW], f32, tag="ps")
                                for j in range(2):
                                    nc.tensor.matmul(
                                        ps[:, j * QW:(j + 1) * QW],
                                        kt_sb[pr][j * KD:(j + 1) * KD,
                                                  sc * P:(sc + 1) * P],
                                        qt_sb[pr][j * KD:(j + 1) * KD,
                                                  qc * QW:(qc + 1) * QW],
                                        start=True, stop=True)
                                on_act = (exp_idx * ACT_FRAC_NUM) % ACT_FRAC_DEN \
                                    < ACT_FRAC_NUM
                                exp_idx += 1
                                if on_act:
                                    et = etp.tile([P, 2 * QW], bf16, tag="et")
                                    nc.scalar.activation(et[:], ps[:], AF.Exp,
                                                         scale=1.0 / EXPA)
                                    etv = et[:]
                                else:
                                    et = etp.tile([P, 2 * QW], i16, tag="et")
                                    nc.vector.tensor_scalar(
                                        et[:], ps[:], EXPB, 0.0,
                                        OP.add, OP.max)
                                    etv = et[:].bitcast(bf16)
                                pair_ets.append(etv)
                            ets[sc] = pair_ets

                        def emit_ctx(sc):
                            for pr in range(NT):
                                etv = ets[sc][pr]
                                for j in range(2):
                                    nc.tensor.matmul(
                                        pc[pr][j * KD:(j + 1) * KD, :],
                                        va_sb[sc][:, (2 * pr + j) * KD:
                                                  (2 * pr + j + 1) * KD],
                                        etv[:, j * QW:(j + 1) * QW],
                                        start=(sc == 0), stop=(sc == NSC - 1))
                            for pr in range(NT):
                                etv = ets[sc][pr]
                                for j in range(2):
                                    r = 32 * (2 * pr + j)
                                    nc.tensor.matmul(
                                        pdn[r:r + 1, :],
                                        ones_bf[:],
                                        etv[:, j * QW:(j + 1) * QW],
                                        start=(sc == 0), stop=(sc == NSC - 1),
                                        tile_position=(0, r))
                            ets[sc] = None

                        for sc in range(NSC):
                            emit_scores(sc)
                            if sc > 0:
                                emit_ctx(sc - 1)
                        emit_ctx(NSC - 1)

                        # normalize: cn = pc * (1/denom) broadcast
                        dn_t = dsbp.tile([P, QW], bf16, tag="dsb")
                        nc.vector.tensor_copy(dn_t[:], pdn[:])
                        for pr in range(NT):
                            prb = dnp.tile([P, QW], f32, tag="dn",
                                           name=f"prb{qc}_{pr}")
                            nc.tensor.matmul(prb[:],
                                             selm[:, pr * P:(pr + 1) * P],
                                             dn_t[:], start=True, stop=True)
                            rb_t = rbp.tile([P, QW], bf16, tag="rb")
                            with nc.allow_low_precision(
                                    reason="1/denom in bf16 is plenty"):
                                nc.vector.reciprocal(rb_t[:], prb[:])
                            cn = cnp.tile([P, QW], bf16, tag="cn")
                            nc.vector.tensor_tensor(cn[:], pc[pr][:], rb_t[:],
                                                    OP.mult)
                            for h in range(2):
                                blk = 2 * qc + h
                                nc.sync.dma_start(
                                    out=a2a_in[blk * 2 * P + pr * P:
                                               blk * 2 * P + (pr + 1) * P, :],
                                    in_=cn[:, h * 2 * P:(h + 1) * 2 * P])

                # ---- AllToAll across all 8 cores ----
                nc.gpsimd.collective_compute(
                    "AllToAll", mybir.AluOpType.bypass,
                    replica_groups=[[0, 1, 2, 3, 4, 5, 6, 7]],
                    ins=[a2a_in[:].opt()],
                    outs=[a2a_out[:].opt()])

                # ---- output projection + residual + LayerNorm ----
                with tc.tile_pool(name="ctxa", bufs=2 * HC) as ctxp, \
                     tc.tile_pool(name="op", bufs=2, space="PSUM") as opp, \
                     tc.tile_pool(name="ln", bufs=2) as lnp, \
                     tc.tile_pool(name="st", bufs=4) as stp:
                    ctx_all = {}
                    for b_ in range(2):
                        for hc in range(HC):
                            t = ctxp.tile([P, 2 * P], bf16, tag="ctxa")
                            src = (4 * b_ + hc // 2) * 2 * P + (hc % 2) * P
                            nc.sync.dma_start(
                                out=t[:], in_=a2a_out[src:src + P, :])
                            ctx_all[(b_, hc)] = t
                    for qs in range(SB // P):
                        b_, q2 = qs // 2, qs % 2
                        po = opp.tile([P, D], f32, tag="op")
                        for d5 in range(D // QW):
                            for hc in range(HC):
                                nc.tensor.matmul(
                                    po[:, d5 * QW:(d5 + 1) * QW],
                                    ctx_all[(b_, hc)][:, q2 * P:(q2 + 1) * P],
                                    wo_sb[hc][:, d5 * QW:(d5 + 1) * QW],
                                    start=(hc == 0), stop=(hc == HC - 1))
                        y = lnp.tile([P, D], bf16, tag="y")
                        nc.vector.tensor_tensor(y[:], po[:], xq_sb[qs][:],
                                                OP.add)
                        sum_t = stp.tile([P, 1], f32, tag="sum")
                        nc.vector.reduce_sum(out=sum_t[:], in_=y[:], axis=AX)
                        mean_t = stp.tile([P, 1], f32, tag="mean")
                        nc.vector.tensor_scalar_mul(mean_t[:], sum_t[:],
                                                    1.0 / D)
                        cent = lnp.tile([P, D], bf16, tag="cent")
                        nc.vector.tensor_scalar(cent[:], y[:], mean_t[:],
                                                None, OP.subtract)
                        sq = lnp.tile([P, D], bf16, tag="sq")
                        vs = stp.tile([P, 1], f32, tag="vs")
                        nc.scalar.activation(sq[:], cent[:], AF.Square,
                                             accum_out=vs[:])
                        std = stp.tile([P, 1], f32, tag="std")
                        nc.scalar.activation(std[:], vs[:], AF.Sqrt,
                                             bias=eps_t[:], scale=1.0 / D)
                        rstd = stp.tile([P, 1], f32, tag="rstd")
                        nc.vector.reciprocal(rstd[:], std[:])
                        z = lnp.tile([P, D], bf16, tag="z")
                        nc.vector.tensor_scalar_mul(z[:], cent[:], rstd[:])
                        zg = lnp.tile([P, D], bf16, tag="zg")
                        nc.vector.tensor_tensor(zg[:], z[:], gmB[:], OP.mult)
                        ot = lnp.tile([P, D], f32, tag="ot")
                        nc.vector.tensor_tensor(ot[:], zg[:], btB[:], OP.add)
                        nc.sync.dma_start(out=out[qs * P:(qs + 1) * P, :],
                                          in_=ot[:])

    # Post-pass: walrus's per-instruction ISA structs hold only ONE sync
    # wait for compute-engine instructions. Move excess waits onto standalone
    # EventSemaphore instructions placed just before on the same engine
    # stream (sequencer executes them in order; semantics unchanged).
    SPLIT = {"InstMatmult", "InstTensorScalarPtr", "InstTensorScalar",
             "InstTensorTensor", "InstReciprocal", "InstActivation",
             "InstTensorReduce", "InstTensorCopy", "InstMemSet",
             "InstCopy", "InstDMACopy", "InstDMATranspose", "InstDrain",
             "InstCollectiveCompute"}
    evt_n = 0
    for f in nc.m.functions:
        for bb in f.blocks:
            need = any(
                type(i).__name__ in SPLIT and i.sync_info is not None
                and len(i.sync_info.on_wait) > 1 for i in bb.instructions)
            if not need:
                continue
            newl = []
            for ins in bb.instructions:
                si = ins.sync_info
                if (type(ins).__name__ in SPLIT and si is not None
                        and len(si.on_wait) > 1):
                    extra = list(si.on_wait[:-1])
                    for j in range(0, len(extra), 2):  # evt-sem holds <=2
                        evt_n += 1
                        evt = mybir.InstEventSemaphore(name=f"mmwait_{evt_n}")
                        evt.engine = ins.engine
                        evt.sync_info = mybir.SyncInfo(
                            on_wait=extra[j:j + 2], on_update=[])
                        newl.append(evt)
                    ins.sync_info = mybir.SyncInfo(
                        on_wait=[si.on_wait[-1]],
                        on_update=list(si.on_update))
                newl.append(ins)
            bb.instructions = newl
    return nc


def get_nc():
    if "nc" not in _cache:
        _cache["nc"] = _build()
    return _cache["nc"]


def make_in_maps(inputs, n_cores=8):
    """Shard full inputs into per-core input maps (host-side layout prep)."""
    from ml_dtypes import bfloat16
    f = np.float32
    HK = H * KD
    qscale = EXPA / np.sqrt(KD)
    wq_ = np.asarray(inputs["wq"], f).reshape(D, HK) * qscale
    wk_ = np.asarray(inputs["wk"], f).reshape(D, HK)
    wv_ = np.asarray(inputs["wv"], f).reshape(D, HK)
    wo_ = np.ascontiguousarray(
        np.asarray(inputs["wo"], f).reshape(HK, D)).astype(bfloat16)
    bq_ = np.asarray(inputs["bq"], f).reshape(HK) * qscale
    bk_ = np.asarray(inputs["bk"], f).reshape(HK)
    bv_ = np.asarray(inputs["bv"], f).reshape(HK)
    bo_ = np.asarray(inputs["bo"], f).reshape(D)
    gm_row = np.asarray(inputs["gamma"], f).reshape(1, D).astype(bfloat16)
    bt_row = np.asarray(inputs["beta"], f).reshape(1, D)
    ident = np.eye(P, dtype=f).astype(bfloat16)
    selm = np.zeros((P, 2 * P), f)
    for pr in range(NT):
        selm[64 * pr, pr * P: pr * P + 64] = 1.0
        selm[64 * pr + 32, pr * P + 64: pr * P + 128] = 1.0
    selm = selm.astype(bfloat16)

    xT_cache, x_cache = {}, {}
    wcache = {}
    maps = []
    for c in range(n_cores):
        b, g = c // 4, c % 4
        if b not in xT_cache:
            xb = np.asarray(inputs["x"][b], f)
            x_cache[b] = xb
            xT_cache[b] = np.ascontiguousarray(xb.T).astype(bfloat16)
        if g not in wcache:
            hk = slice(g * GHK, (g + 1) * GHK)
            wcache[g] = dict(
                wq=np.ascontiguousarray(wq_[:, hk]).astype(bfloat16),
                wk=np.ascontiguousarray(wk_[:, hk]).astype(bfloat16),
                wv=np.ascontiguousarray(wv_[:, hk]).astype(bfloat16),
                bqT=np.ascontiguousarray(bq_[hk].reshape(NT, P).T),
                bkT=np.ascontiguousarray(bk_[hk].reshape(NT, P).T),
                bvT=np.ascontiguousarray(bv_[hk].reshape(NT, P).T),
            )
        # output rows [256c, 256c+256) of BOTH batches (residual+bias input)
        xqb = np.concatenate([
            x_cache.setdefault(
                bb, np.asarray(inputs["x"][bb], f))[
                    256 * c:256 * (c + 1)] + bo_
            for bb in range(2)], axis=0)
        maps.append(dict(
            xT=xT_cache[b],
            wo=wo_,
            xqb=xqb,
            gamma_row=gm_row, beta_row=bt_row,
            ident=ident, selm=selm,
            **wcache[g],
        ))
    return maps


def assemble(res):
    outp = np.empty((B, S, D), np.float32)
    for c in range(8):
        o = res.results[c]["out"]
        for b_ in range(2):
            outp[b_, 256 * c:256 * (c + 1)] = o[256 * b_:256 * (b_ + 1)]
    return outp


def kernel(**inputs):
    from concourse.bass_utils import run_bass_kernel_spmd
    nc = get_nc()
    maps = make_in_maps(inputs)
    res = run_bass_kernel_spmd(nc, maps, list(range(8)))
    return assemble(res)
